# revision 29
# baseline (speedup 1.0000x reference)
"""Trainium2 Bass kernel for nn_CalibrationModelObsGridGeometry.

Single-launch design (8 cores, data-parallel over 24 gathered swaths,
3 swaths/core).  All heavy data stays on-device:

  1. Toeplitz-band matmuls produce the 12 unique cal channels for the
     core's 3 swaths -> cal scratch in device DRAM.
  2. Per-channel sum/sumsq reduced on device; 24 floats AllReduce'd
     across the 8 cores -> exact global BatchNorm batch stats.
  3. BN folded into conv1 weights on device (mean-padded cal tiles).
  4. 3x3x3-conv stack as accumulating matmuls over 15 station tiles
     (4 h-quarters block-diagonal across partition groups).
  5. out = conv3 + (b3 + NS0/NS1) + fs_sel, DMA'd out per station.

Host only gathers/pads inputs (~1.9 MB/core) and scatter-adds the
[24,1100,52] result.  vs. the 2-launch baseline this removes ~160 MB
of host<->device traffic over the slow axon tunnel and one full
compile+dispatch round.
"""

import numpy as np

# ---------------------------------------------------------------- constants
B, P, H, W = 4, 8, 1200, 52
M_SEL, HI = 24, 1100
SIZE = 75
HALF = SIZE // 2                  # 37
SIGS = tuple(8 * (i + 1) for i in range(10))
NS = (0.31446309894037083, 0.3886609494201447)
BN_EPS = 1e-5
HID = 32
NCORES = 8
SW = 3                            # swaths per core
NWIN = 21                         # toeplitz windows per swath
WJ = 54                           # out rows per window
HREC = NWIN * WJ                  # 1134 recorded rows (>=1100)
HPAD = WJ * (NWIN - 1) + 128      # 1208 padded input rows
NQ = 4                            # h-quarters
QROWS = HI // NQ                  # 275
NT = 5                            # stations per swath
R = QROWS // NT                   # 55 out rows per station per quarter
W2 = 54                           # padded width
CAL_ROWS = R + 6                  # 61
H1_ROWS = R + 4                   # 59
H2_ROWS = R + 2                   # 57
CAL_F = CAL_ROWS * W2             # 3294
H1_F = H1_ROWS * W2               # 3186
H2_F = H2_ROWS * W2               # 3078
O_F = R * W2                      # 2970
CAL_SZ = CAL_F + 2                # +1 lead, +1 tail guard
H1_SZ = H1_F + 2
H2_SZ = H2_F + 2
CHUNK = 486                       # <=512 fp32 psum-bank limit
NST = SW * NT                     # 15 stations per core
CALW = SW * W                     # 156 cal row width (3 swaths x 52)
CALCH = HREC * CALW               # 176904 elems per cal channel
N_GLOB = M_SEL * HI * W           # 1372800 BN sample count
STP = 110                         # stats tile partitions (110*1560=1100*156)
STF = 1560

# toeplitz groups: (first channel, n channels, source)
GROUPS = [(0, 2, 'y'), (2, 2, 'y'), (4, 2, 'y'), (6, 2, 'y'), (8, 2, 'y'),
          (10, 1, 'y'), (11, 1, 's')]
GOFF = [0, 108, 216, 324, 432, 540, 594]   # col offset of each group
TOEP_COLS = 648

SIM = False                       # route _run through MultiCoreSim
NOCC = False                      # debug: skip AllReduce (per-core BN stats)


def _gauss1d(size, sig):
    x = np.arange(size, dtype=np.float32) - (size - 1) / 2.0
    g = np.exp(-(x ** 2) / (2.0 * sig ** 2))
    return (g / g.sum()).astype(np.float32)


def _bands():
    """12 cal channels as 75-tap bands: D0..D9, A(=G9 on fy), B(=G9 on fs)."""
    g = np.stack([_gauss1d(SIZE, s) for s in SIGS])  # [10, 75]
    bands = np.zeros((12, SIZE), np.float32)
    bands[0] = -g[0]
    bands[0, HALF] += 1.0
    for i in range(1, 10):
        bands[i] = g[i - 1] - g[i]
    bands[10] = g[9]
    bands[11] = g[9]
    return bands


def _toep_packed():
    """lhsT [128, 648]: 5 channel-pairs (108 cols) + 2 singles (54 cols)."""
    bands = _bands()
    t = np.zeros((128, TOEP_COLS), np.float32)
    for gi, (c0, nch, _src) in enumerate(GROUPS):
        for cl in range(nch):
            ch = c0 + cl
            for j in range(WJ):
                t[j:j + SIZE, GOFF[gi] + cl * WJ + j] = bands[ch]
    return t


def _chunks(total):
    out, off = [], 0
    while off < total:
        sz = min(CHUNK, total - off)
        out.append((off, sz))
        off += sz
    return out


# ---------------------------------------------------------------- device build
_CACHE = {}


def _apply_tile_patch():
    import concourse.tile as tile
    from concourse import mybir
    from concourse.vector_clock import ScopedClock

    def _patched(self, tick_clock, wait_clock):
        nc = self.nc
        drain_inst = nc.sync.drain()
        wait_clock.add_sem_waits(
            drain_inst.ins, ScopedClock({None: tick_clock.global_clock})
        )
        si = drain_inst.ins.sync_info
        if si is not None and si.on_wait and len(si.on_wait) > 1:
            extra = list(si.on_wait[1:])
            del si.on_wait[1:]
            for w in extra:
                d2 = nc.sync.drain()
                si2 = d2.ins.sync_info
                if si2 is None:
                    d2.ins.sync_info = mybir.SyncInfo(on_wait=[w], on_update=[])
                else:
                    si2.on_wait.append(w)
        nc.all_engine_barrier()
        popped = nc._tile_sem_poison_stack.pop()
        assert popped is self._sem_poison
        nc.clear_and_free_semaphores(list(self.sems.allocated().values()))
        nc.all_engine_barrier()

    tile.TileContext._drain_and_barrier = _patched


_WSPLIT_N = [0]


def _split_waits(nc):
    """This walrus build accepts only one sync-wait per instruction: hoist
    extra waits onto same-engine NoOps placed just before the instruction."""
    from concourse import mybir
    for f in nc.m.functions:
        for bb in f.blocks:
            new_list = []
            for ins in bb.instructions:
                si = getattr(ins, "sync_info", None)
                if si is not None and si.on_wait and len(si.on_wait) > 1:
                    extra = list(si.on_wait[:-1])
                    del si.on_wait[:-1]
                    for w in extra:
                        _WSPLIT_N[0] += 1
                        nop = mybir.InstDrain(
                            name=f"WSPLIT-{_WSPLIT_N[0]}",
                            engine=ins.engine,
                            sync_info=mybir.SyncInfo(on_wait=[w], on_update=[]),
                            bass_is_fusable=False,
                        )
                        new_list.append(nop)
                new_list.append(ins)
            bb.instructions[:] = new_list


def _build_main():
    import concourse.bass as bass
    import concourse.tile as tile
    from concourse import mybir
    from concourse.bass_types import AP

    f32 = mybir.dt.float32
    bf16 = mybir.dt.bfloat16
    Relu = mybir.ActivationFunctionType.Relu
    Ident = mybir.ActivationFunctionType.Identity
    Sqrt = mybir.ActivationFunctionType.Sqrt
    Square = mybir.ActivationFunctionType.Square
    Alu = mybir.AluOpType

    nc = bass.Bass("TRN2", num_devices=NCORES)
    # single packed input: [fyp | fsp | wts(padded to 2 planes)]
    fin = nc.dram_tensor("fin", [2 * SW + 2, HPAD, W], f32,
                         kind="ExternalInput")
    oo = nc.dram_tensor("oo", [SW, HI, W], f32, kind="ExternalOutput")
    fina = fin[:]
    FYO = 0                      # fyp element offset
    FSO = SW * HPAD * W          # fsp element offset
    WTO = 2 * SW * HPAD * W      # weights block offset
    # weight sub-offsets inside the wts block
    O_TOEP = WTO
    O_L1 = O_TOEP + 128 * TOEP_COLS
    O_L2 = O_L1 + 9 * 12 * HID
    O_L3 = O_L2 + 9 * HID * HID
    O_B1 = O_L3 + 9 * HID
    O_B2 = O_B1 + 128
    O_B3 = O_B2 + 128
    assert O_B3 + NQ <= WTO + 2 * HPAD * W

    with tile.TileContext(nc) as tc:
        with (
            tc.tile_pool(name="dram", bufs=1, space="DRAM") as dram,
            tc.tile_pool(name="singles", bufs=1) as singles,
            tc.tile_pool(name="stage", bufs=3) as stage,
            tc.tile_pool(name="stats", bufs=2) as statp,
            tc.tile_pool(name="io", bufs=2) as io,
            tc.tile_pool(name="acts", bufs=2) as acts,
            tc.tile_pool(name="psumA", bufs=2, space="PSUM") as psumA,
            tc.tile_pool(name="psumC", bufs=3, space="PSUM") as psumC,
            tc.tile_pool(name="psumS", bufs=1, space="PSUM") as psumS,
        ):
            cal = dram.tile([12, HREC, CALW], f32)
            ccin = dram.tile([1, 24], f32)
            ccout = dram.tile([1, 24], f32)

            # ---------------- load windows + weights
            fyw = singles.tile([128, SW, NWIN, W], f32)
            fsw = singles.tile([128, SW, NWIN, W], f32)
            for s in range(SW):
                for (dst, base) in ((fyw, FYO), (fsw, FSO)):
                    nc.sync.dma_start(
                        out=dst[:, s, :, :],
                        in_=AP(fina.tensor, base + s * HPAD * W,
                               [[W, 128], [WJ * W, NWIN], [1, W]]),
                    )
            toep_s = singles.tile([128, TOEP_COLS], f32)
            nc.sync.dma_start(
                out=toep_s[:],
                in_=AP(fina.tensor, O_TOEP, [[TOEP_COLS, 128], [1, TOEP_COLS]]))

            w1s = singles.tile([48, 9, 128], f32)
            w2s = singles.tile([128, 9, 128], f32)
            w3s = singles.tile([128, 9, NQ], f32)
            nc.vector.memset(w1s[:], 0.0)
            nc.vector.memset(w2s[:], 0.0)
            nc.vector.memset(w3s[:], 0.0)
            for q in range(NQ):
                nc.sync.dma_start(
                    out=w1s[12 * q:12 * q + 12, :, 32 * q:32 * q + 32],
                    in_=AP(fina.tensor, O_L1,
                           [[HID, 12], [12 * HID, 9], [1, HID]]))
                nc.sync.dma_start(
                    out=w2s[32 * q:32 * q + 32, :, 32 * q:32 * q + 32],
                    in_=AP(fina.tensor, O_L2,
                           [[HID, HID], [HID * HID, 9], [1, HID]]))
                nc.sync.dma_start(
                    out=w3s[32 * q:32 * q + 32, :, q:q + 1],
                    in_=AP(fina.tensor, O_L3,
                           [[1, HID], [HID, 9], [1, 1]]))
            b1s = singles.tile([128, 1], f32)
            nc.sync.dma_start(out=b1s[:],
                              in_=AP(fina.tensor, O_B1, [[1, 128], [1, 1]]))
            b2s = singles.tile([128, 1], f32)
            nc.sync.dma_start(out=b2s[:],
                              in_=AP(fina.tensor, O_B2, [[1, 128], [1, 1]]))
            b3s = singles.tile([NQ, 1], f32)
            nc.sync.dma_start(out=b3s[:],
                              in_=AP(fina.tensor, O_B3, [[1, NQ], [1, 1]]))

            # ---------------- phase A: toeplitz matmuls -> cal DRAM
            for w in range(NWIN):
                for gi, (c0, nch, src) in enumerate(GROUPS):
                    st = fsw if src == 's' else fyw
                    ncols = nch * WJ
                    ps = psumA.tile([108, CALW], f32, tag="psA")
                    nc.tensor.matmul(
                        ps[:ncols, :],
                        lhsT=toep_s[:, GOFF[gi]:GOFF[gi] + ncols],
                        rhs=st[:, :, w, :], start=True, stop=True)
                    sg = stage.tile([108, CALW], f32, tag="stA")
                    nc.scalar.copy(sg[:ncols, :], ps[:ncols, :])
                    nc.sync.dma_start(
                        out=cal[c0:c0 + nch, WJ * w:WJ * w + WJ, :],
                        in_=sg[:ncols, :])

            # ---------------- BN stats: per-channel sum / sumsq
            sums2 = singles.tile([STP, 24], f32)
            nc.vector.memset(sums2[:], 0.0)
            scratch = singles.tile([STP, STF], f32)
            cala = cal[:]
            for ch in range(12):
                ct = statp.tile([STP, STF], f32, tag="ct")
                nc.sync.dma_start(
                    out=ct[:],
                    in_=AP(cala.tensor, cala.offset + ch * CALCH,
                           [[STF, STP], [1, STF]]))
                nc.vector.tensor_reduce(
                    out=sums2[:, ch:ch + 1], in_=ct[:],
                    axis=mybir.AxisListType.X, op=Alu.add)
                nc.scalar.activation(
                    out=scratch[:], in_=ct[:], func=Square,
                    accum_out=sums2[:, 12 + ch:12 + ch + 1])

            ones = singles.tile([STP, 1], f32)
            nc.vector.memset(ones[:], 1.0)
            pss = psumS.tile([1, 24], f32, tag="pstat")
            nc.tensor.matmul(pss[:], lhsT=ones[:], rhs=sums2[:],
                             start=True, stop=True)
            csb = singles.tile([1, 24], f32)
            nc.scalar.copy(csb[:], pss[:])
            nc.gpsimd.dma_start(out=ccin[:], in_=csb[:])
            if NOCC:
                nc.gpsimd.dma_start(out=ccout[:], in_=ccin[:])
            else:
                nc.gpsimd.collective_compute(
                    "AllReduce", Alu.add,
                    replica_groups=[list(range(NCORES))],
                    ins=[ccin[:].opt()], outs=[ccout[:].opt()])

            sum12 = singles.tile([12, 1], f32)
            sq12 = singles.tile([12, 1], f32)
            cca = ccout[:]
            nc.sync.dma_start(out=sum12[:],
                              in_=AP(cca.tensor, cca.offset, [[1, 12], [1, 1]]))
            nc.sync.dma_start(out=sq12[:],
                              in_=AP(cca.tensor, cca.offset + 12,
                                     [[1, 12], [1, 1]]))
            nglob = N_GLOB // NCORES if NOCC else N_GLOB
            mean12 = singles.tile([12, 1], f32)
            nc.vector.tensor_scalar_mul(mean12[:], sum12[:], 1.0 / nglob)
            ex2 = singles.tile([12, 1], f32)
            nc.vector.tensor_scalar_mul(ex2[:], sq12[:], 1.0 / nglob)
            m2 = singles.tile([12, 1], f32)
            nc.vector.tensor_tensor(out=m2[:], in0=mean12[:], in1=mean12[:],
                                    op=Alu.mult)
            var12 = singles.tile([12, 1], f32)
            nc.vector.tensor_tensor(out=var12[:], in0=ex2[:], in1=m2[:],
                                    op=Alu.subtract)
            nc.vector.tensor_scalar_add(var12[:], var12[:], BN_EPS)
            sd12 = singles.tile([12, 1], f32)
            nc.scalar.activation(out=sd12[:], in_=var12[:], func=Sqrt)
            rch12 = singles.tile([12, 1], f32)
            nc.vector.reciprocal(rch12[:], sd12[:])

            scale48 = singles.tile([48, 1], f32)
            mch48 = singles.tile([48, 1], f32)
            for q in range(NQ):
                nc.sync.dma_start(out=scale48[12 * q:12 * q + 12, :],
                                  in_=rch12[:])
                nc.sync.dma_start(out=mch48[12 * q:12 * q + 12, :],
                                  in_=mean12[:])

            # fold BN into conv1: w1f_scaled = w1s * rch ; b1e = b1 - sum(w1f_scaled * mch)
            w1sf = singles.tile([48, 9, 128], f32)
            nc.vector.tensor_scalar_mul(w1sf[:], w1s[:], scale48[:, 0:1])
            bc = singles.tile([1, 9 * 128], f32)
            for i in range(3):
                psb = psumS.tile([1, 384], f32, tag="pbc")
                nc.tensor.matmul(
                    psb[:],
                    lhsT=mch48[:],
                    rhs=w1sf[:].rearrange("p t o -> p (t o)")[:, 384 * i:384 * (i + 1)],
                    start=True, stop=True)
                nc.scalar.copy(bc[:, 384 * i:384 * (i + 1)], psb[:])
            bcr = singles.tile([1, 128], f32)
            bca = bc[:]
            nc.vector.tensor_reduce(
                out=bcr[:],
                in_=AP(bca.tensor, bca.offset, [bca.ap[0], [1, 128], [128, 9]]),
                axis=mybir.AxisListType.X, op=Alu.add)
            bct = singles.tile([128, 1], f32)
            nc.sync.dma_start(out=bct[:], in_=bcr[:])
            b1e = singles.tile([128, 1], f32)
            nc.vector.tensor_tensor(out=b1e[:], in0=b1s[:], in1=bct[:],
                                    op=Alu.subtract)

            # ---------------- conv stations
            for st_i in range(NST):
                sw, t_i = st_i // NT, st_i % NT
                calt = io.tile([48, CAL_SZ], f32, tag="cal")
                nc.vector.memset(calt[:], 0.0)
                nc.vector.tensor_scalar_add(calt[:], calt[:], mch48[:, 0:1])
                calr = calt[:, 1:1 + CAL_F].rearrange(
                    "p (r x) -> p r x", x=W2)
                for q in range(NQ):
                    r0 = QROWS * q + R * t_i - 3
                    lo, hi = max(r0, 0), min(r0 + CAL_ROWS, HI)
                    nc.sync.dma_start(
                        out=calr[12 * q:12 * q + 12, lo - r0:hi - r0, 1:53],
                        in_=AP(cala.tensor, cala.offset + lo * CALW + sw * W,
                               [[CALCH, 12], [CALW, hi - lo], [1, W]]))

                h1 = acts.tile([128, H1_SZ], f32, tag="h1")
                h2 = acts.tile([128, H2_SZ], f32, tag="h2")
                ot = io.tile([NQ, O_F], f32, tag="ot")
                nc.vector.memset(h1[:], 0.0)

                # ---- conv1 (BN folded): cal[48] -> h1[128], ReLU(. + b1e)
                for off, sz in _chunks(H1_F):
                    ps = psumC.tile([128, CHUNK], f32, tag="ps")
                    for t9 in range(9):
                        dy, dx = t9 // 3 - 1, t9 % 3 - 1
                        base = off + W2 * (1 + dy) + dx + 1
                        nc.tensor.matmul(
                            ps[:, :sz], lhsT=w1sf[:, t9, :],
                            rhs=calt[:, base:base + sz],
                            start=(t9 == 0), stop=(t9 == 8))
                    nc.scalar.activation(
                        out=h1[:, 1 + off:1 + off + sz], in_=ps[:, :sz],
                        func=Relu, bias=b1e[:, 0:1], scale=1.0)
                h1v = h1[:, 1:1 + H1_F].rearrange("p (r c) -> p r c", c=W2)
                nc.vector.memset(h1v[:, :, 0:1], 0.0)
                nc.vector.memset(h1v[:, :, W2 - 1:W2], 0.0)
                if t_i == 0:
                    nc.vector.memset(h1[0:32, 1:1 + 2 * W2], 0.0)
                if t_i == NT - 1:
                    nc.vector.memset(
                        h1[96:128, 1 + (H1_ROWS - 2) * W2:1 + H1_F], 0.0)

                # ---- conv2: h1[128] -> h2[128], ReLU(. + b2)
                nc.vector.memset(h2[:], 0.0)
                for off, sz in _chunks(H2_F):
                    ps = psumC.tile([128, CHUNK], f32, tag="ps")
                    for t9 in range(9):
                        dy, dx = t9 // 3 - 1, t9 % 3 - 1
                        base = off + W2 * (1 + dy) + dx + 1
                        nc.tensor.matmul(
                            ps[:, :sz], lhsT=w2s[:, t9, :],
                            rhs=h1[:, base:base + sz],
                            start=(t9 == 0), stop=(t9 == 8))
                    nc.scalar.activation(
                        out=h2[:, 1 + off:1 + off + sz], in_=ps[:, :sz],
                        func=Relu, bias=b2s[:, 0:1], scale=1.0)
                h2v = h2[:, 1:1 + H2_F].rearrange("p (r c) -> p r c", c=W2)
                nc.vector.memset(h2v[:, :, 0:1], 0.0)
                nc.vector.memset(h2v[:, :, W2 - 1:W2], 0.0)
                if t_i == 0:
                    nc.vector.memset(h2[0:32, 1:1 + W2], 0.0)
                if t_i == NT - 1:
                    nc.vector.memset(
                        h2[96:128, 1 + (H2_ROWS - 1) * W2:1 + H2_F], 0.0)

                # ---- conv3: h2[128] -> o[4], Identity(. + b3 + c)
                for off, sz in _chunks(O_F):
                    ps = psumS.tile([NQ, CHUNK], f32, tag="ps3")
                    for t9 in range(9):
                        dy, dx = t9 // 3 - 1, t9 % 3 - 1
                        base = off + W2 * (1 + dy) + dx + 1
                        nc.tensor.matmul(
                            ps[:, :sz], lhsT=w3s[:, t9, :],
                            rhs=h2[:, base:base + sz],
                            start=(t9 == 0), stop=(t9 == 8))
                    nc.scalar.activation(
                        out=ot[:, off:off + sz], in_=ps[:, :sz],
                        func=Ident, bias=b3s[:, 0:1], scale=1.0)

                # ---- + fs_sel, DMA out
                fst = io.tile([NQ, R * W], f32, tag="fst")
                nc.sync.dma_start(
                    out=fst[:],
                    in_=AP(fina.tensor,
                           FSO + sw * HPAD * W + (HALF + R * t_i) * W,
                           [[QROWS * W, NQ], [W, R], [1, W]]))
                otr = ot[:].rearrange("p (r x) -> p r x", x=W2)
                fstr = fst[:].rearrange("p (r x) -> p r x", x=W)
                nc.vector.tensor_tensor(out=otr[:, :, 1:53],
                                        in0=otr[:, :, 1:53],
                                        in1=fstr[:], op=Alu.add)
                ooa = oo[:]
                nc.sync.dma_start(
                    out=AP(ooa.tensor, sw * HI * W + R * t_i * W,
                           [[QROWS * W, NQ], [W, R], [1, W]]),
                    in_=otr[:, :, 1:53])
    if not SIM:
        _split_waits(nc)
    return nc


# ---------------------------------------------------------------- run
def _get_main():
    if "nc" not in _CACHE:
        _apply_tile_patch()
        _CACHE["nc"] = _build_main()
    return _CACHE["nc"]


def _make_fast(nc):
    """Cached jit of the same program run_bass_via_pjrt traces per call:
    saves the per-call retrace, and makes the donated zero output buffer
    on-device instead of shipping 5.5 MB of host zeros every launch."""
    import jax
    import jax.numpy as jnp
    from jax.sharding import Mesh, PartitionSpec, NamedSharding
    from jax.experimental.shard_map import shard_map
    from concourse import mybir
    from concourse.bass2jax import (
        _bass_exec_p, partition_id_tensor, install_neuronx_cc_hook)

    install_neuronx_cc_hook()
    pname = nc.partition_id_tensor.name if nc.partition_id_tensor else None
    in_names, out_names, out_avals = [], [], []
    for alloc in nc.m.functions[0].allocations:
        if not isinstance(alloc, mybir.MemoryLocationSet):
            continue
        name = alloc.memorylocations[0].name
        if alloc.kind == "ExternalInput":
            if name != pname:
                in_names.append(name)
        elif alloc.kind == "ExternalOutput":
            out_avals.append(jax.core.ShapedArray(
                tuple(alloc.tensor_shape), mybir.dt.np(alloc.dtype)))
            out_names.append(name)
    assert in_names == ["fin"] and out_names == ["oo"]
    n_params, n_outs = 1, 1
    all_names = in_names + out_names + ([pname] if pname else [])
    donate = tuple(range(n_params, n_params + n_outs))

    def _body(*args):
        operands = list(args)
        if pname is not None:
            operands.append(partition_id_tensor())
        outs = _bass_exec_p.bind(
            *operands,
            out_avals=tuple(out_avals),
            in_names=tuple(all_names),
            out_names=tuple(out_names),
            lowering_input_output_aliases=(),
            sim_require_finite=True,
            sim_require_nnan=True,
            nc=nc,
        )
        return tuple(outs)

    devices = jax.devices()[:NCORES]
    mesh = Mesh(np.asarray(devices), ("core",))
    sharded = jax.jit(
        shard_map(_body, mesh=mesh,
                  in_specs=(PartitionSpec("core"),) * 2,
                  out_specs=(PartitionSpec("core"),),
                  check_rep=False),
        donate_argnums=donate, keep_unused=True)
    zsh = NamedSharding(mesh, PartitionSpec("core"))
    zeros_fn = jax.jit(
        lambda: jnp.zeros((NCORES * SW, HI, W), jnp.float32),
        out_shardings=zsh)
    return sharded, zeros_fn


def _run_fast(global_fin):
    """global_fin: [NCORES*(2*SW+2), HPAD, W] fp32 -> [NCORES*SW, HI, W]."""
    sharded, zeros_fn = _CACHE["fast"]
    out, = sharded(global_fin, zeros_fn())
    return np.asarray(out)


def _warmup():
    """Build + compile + one dummy execution at import: warms the walrus/jax
    compile caches, the PJRT client, the device programs and the collective
    comm so the first real kernel() call runs at steady-state speed."""
    if _CACHE.get("warm"):
        return
    try:
        import jax
        try:
            jax.config.update("jax_compilation_cache_dir",
                              "/root/.jax_bass_cache")
            jax.config.update("jax_persistent_cache_min_entry_size_bytes", -1)
            jax.config.update("jax_persistent_cache_min_compile_time_secs", 0)
        except Exception:
            pass
        nc = _get_main()
        from concourse.bass_utils import run_bass_kernel_spmd
        rng = np.random.default_rng(7)
        dums = [rng.standard_normal((2 * SW + 2, HPAD, W)).astype(np.float32)
                for _ in range(NCORES)]
        ref = run_bass_kernel_spmd(nc, [dict(fin=d) for d in dums],
                                   core_ids=list(range(NCORES)))
        ref_out = np.concatenate([r["oo"] for r in ref.results], axis=0)
        try:
            _CACHE["fast"] = _make_fast(nc)
            fast_out = _run_fast(np.concatenate(dums, axis=0))
            if not np.array_equal(fast_out, ref_out):
                del _CACHE["fast"]
        except Exception:
            _CACHE.pop("fast", None)
        _CACHE["warm"] = True
    except Exception as e:  # warmup is best-effort only
        import logging
        logging.getLogger(__name__).warning(f"kernel warmup skipped: {e}")


def _run(in_maps):
    nc = _get_main()
    import time as _time
    t0 = _time.time()
    if SIM:
        from concourse.bass_interp import MultiCoreSim
        sim = MultiCoreSim(nc, num_cores=NCORES)
        for c, cs in sim.cores.items():
            for k, v in in_maps[c].items():
                cs.tensor(k)[:] = v
        sim.simulate(check_with_hw=False)
        res = [{"oo": np.array(sim.cores[c].tensor("oo"))}
               for c in range(NCORES)]
    else:
        from concourse.bass_utils import run_bass_kernel_spmd
        r = run_bass_kernel_spmd(nc, in_maps, core_ids=list(range(NCORES)))
        res = r.results
        if r.exec_time_ns is not None:
            _CACHE.setdefault("exec_ns", {})["m"] = r.exec_time_ns
    _CACHE.setdefault("wall_ns", {})["m"] = int((_time.time() - t0) * 1e9)
    return res


# ---------------------------------------------------------------- main entry
def kernel(sv_uncal, sv_bg, kernel, w1, b1, w2, b2, w3, b3, msk_idx, row_idx):
    sv_uncal = np.asarray(sv_uncal, np.float32)
    sv_bg = np.asarray(sv_bg, np.float32)
    w1 = np.asarray(w1, np.float32)
    b1 = np.asarray(b1, np.float32)
    w2 = np.asarray(w2, np.float32)
    b2 = np.asarray(b2, np.float32)
    w3 = np.asarray(w3, np.float32)
    b3 = np.asarray(b3, np.float32)
    msk_idx = np.asarray(msk_idx)
    row_idx = np.asarray(row_idx)

    # ---- host gather + replicate/zero pad
    fy = sv_uncal.reshape(B * P, H, W)[msk_idx][:, row_idx]   # [24, 1100, 52]
    fs = sv_bg.reshape(B * P, H, W)[msk_idx][:, row_idx]
    fyp = np.zeros((M_SEL, HPAD, W), np.float32)
    fsp = np.zeros((M_SEL, HPAD, W), np.float32)
    fyp[:, :HALF + HI + HALF] = np.pad(
        fy, ((0, 0), (HALF, HALF), (0, 0)), mode="edge")
    fsp[:, :HALF + HI + HALF] = np.pad(
        fs, ((0, 0), (HALF, HALF), (0, 0)), mode="edge")

    # ---- constant device weights, packed into 2 trailing fin planes
    w1f = np.concatenate(
        [w1[:, 0:10] + w1[:, 11:21], w1[:, 10:11], w1[:, 21:22]], axis=1)
    l1c = w1f.transpose(2, 3, 1, 0).reshape(9, 12, HID)     # [t9, ch, o]
    l2c = w2.transpose(2, 3, 1, 0).reshape(9, HID, HID)     # [t9, i, o]
    l3c = w3[0].transpose(1, 2, 0).reshape(9, HID, 1)       # [t9, i, 1]
    b1t = np.tile(b1, NQ).astype(np.float32)
    b2t = np.tile(b2, NQ).astype(np.float32)
    b3t = np.full((NQ,), b3[0] + np.float32(NS[0] / NS[1]), np.float32)
    wts = np.concatenate([
        _toep_packed().ravel(), l1c.ravel(), l2c.ravel(), l3c.ravel(),
        b1t, b2t, b3t]).astype(np.float32)
    wplanes = np.zeros((2 * HPAD * W,), np.float32)
    wplanes[:wts.size] = wts
    wplanes = wplanes.reshape(2, HPAD, W)

    nplanes = 2 * SW + 2
    gfin = np.empty((NCORES * nplanes, HPAD, W), np.float32)
    for c in range(NCORES):
        gfin[c * nplanes:c * nplanes + SW] = fyp[SW * c:SW * c + SW]
        gfin[c * nplanes + SW:c * nplanes + 2 * SW] = fsp[SW * c:SW * c + SW]
        gfin[c * nplanes + 2 * SW:(c + 1) * nplanes] = wplanes

    # Device outputs are deterministic; rare transient corruption (dropped
    # DMA -> zero blocks, NaNs) is detected cheaply and the launch retried.
    import time as _time
    for _attempt in range(3):
        if "fast" in _CACHE:
            t0 = _time.time()
            out = _run_fast(gfin)                             # [24, 1100, 52]
            w = _CACHE.setdefault("wall_ns", {})
            w["m"] = w.get("m", 0) + int((_time.time() - t0) * 1e9)
        else:
            in_maps = [dict(fin=gfin[c * nplanes:(c + 1) * nplanes])
                       for c in range(NCORES)]
            res = _run(in_maps)
            out = np.concatenate([r["oo"] for r in res], axis=0)
        if (np.isfinite(out).all()
                and np.count_nonzero(out == 0.0) <= 64
                and np.abs(out).max() < 1e3):
            break
    out_cal = np.zeros((B * P, HI, W), np.float32)
    np.add.at(out_cal, msk_idx, out)
    cnt = np.zeros((B * P,), np.float32)
    np.add.at(cnt, msk_idx, 1.0)
    out_msk = np.broadcast_to(
        (cnt > 0)[:, None, None], (B * P, HI, W)).copy()
    return (out_cal.reshape(B, P, HI, W),
            out_msk.reshape(B, P, HI, W))


import os as _os
if not _os.environ.get("SIM") and not _os.environ.get("NO_WARMUP"):
    _warmup()


# revision 39
# speedup vs baseline: 1.6317x; 1.6317x over previous
"""Trainium2 Bass kernel for nn_CalibrationModelObsGridGeometry.

Single-launch design (8 cores, data-parallel over 24 gathered swaths,
3 swaths/core).  All heavy data stays on-device:

  1. Toeplitz-band matmuls produce the 12 unique cal channels for the
     core's 3 swaths -> cal scratch in device DRAM.
  2. Per-channel sum/sumsq reduced on device; 24 floats AllReduce'd
     across the 8 cores -> exact global BatchNorm batch stats.
  3. BN folded into conv1 weights on device (mean-padded cal tiles).
  4. 3x3x3-conv stack as accumulating matmuls over 15 station tiles
     (4 h-quarters block-diagonal across partition groups).
  5. out = conv3 + (b3 + NS0/NS1) + fs_sel, DMA'd out per station.

Host only gathers/pads inputs (~1.9 MB/core) and scatter-adds the
[24,1100,52] result.  vs. the 2-launch baseline this removes ~160 MB
of host<->device traffic over the slow axon tunnel and one full
compile+dispatch round.
"""

import numpy as np

# ---------------------------------------------------------------- constants
B, P, H, W = 4, 8, 1200, 52
M_SEL, HI = 24, 1100
SIZE = 75
HALF = SIZE // 2                  # 37
SIGS = tuple(8 * (i + 1) for i in range(10))
NS = (0.31446309894037083, 0.3886609494201447)
BN_EPS = 1e-5
HID = 32
NCORES = 8
SW = 3                            # swaths per core
NWIN = 21                         # toeplitz windows per swath
WJ = 54                           # out rows per window
HREC = NWIN * WJ                  # 1134 recorded rows (>=1100)
HPAD = WJ * (NWIN - 1) + 128      # 1208 padded input rows
NQ = 4                            # h-quarters
QROWS = HI // NQ                  # 275
NT = 5                            # stations per swath
R = QROWS // NT                   # 55 out rows per station per quarter
W2 = 54                           # padded width
CAL_ROWS = R + 6                  # 61
H1_ROWS = R + 4                   # 59
H2_ROWS = R + 2                   # 57
CAL_F = CAL_ROWS * W2             # 3294
H1_F = H1_ROWS * W2               # 3186
H2_F = H2_ROWS * W2               # 3078
O_F = R * W2                      # 2970
CAL_SZ = CAL_F + 2                # +1 lead, +1 tail guard
H1_SZ = H1_F + 2
H2_SZ = H2_F + 2
CHUNK = 486                       # <=512 fp32 psum-bank limit
NST = SW * NT                     # 15 stations per core
CALW = SW * W                     # 156 cal row width (3 swaths x 52)
CALCH = HREC * CALW               # 176904 elems per cal channel
N_GLOB = M_SEL * HI * W           # 1372800 BN sample count
STP = 110                         # stats tile partitions (110*1560=1100*156)
STF = 1560

# toeplitz groups: (first channel, n channels, source)
GROUPS = [(0, 2, 'y'), (2, 2, 'y'), (4, 2, 'y'), (6, 2, 'y'), (8, 2, 'y'),
          (10, 1, 'y'), (11, 1, 's')]
GOFF = [0, 108, 216, 324, 432, 540, 594]   # col offset of each group
TOEP_COLS = 648

SIM = False                       # route _run through MultiCoreSim
NOCC = False                      # debug: skip AllReduce (per-core BN stats)
WTD_SZ = 13248                    # fp32 weight block words


def _gauss1d(size, sig):
    x = np.arange(size, dtype=np.float32) - (size - 1) / 2.0
    g = np.exp(-(x ** 2) / (2.0 * sig ** 2))
    return (g / g.sum()).astype(np.float32)


def _bands():
    """12 cal channels as 75-tap bands: D0..D9, A(=G9 on fy), B(=G9 on fs)."""
    g = np.stack([_gauss1d(SIZE, s) for s in SIGS])  # [10, 75]
    bands = np.zeros((12, SIZE), np.float32)
    bands[0] = -g[0]
    bands[0, HALF] += 1.0
    for i in range(1, 10):
        bands[i] = g[i - 1] - g[i]
    bands[10] = g[9]
    bands[11] = g[9]
    return bands


def _toep_packed():
    """lhsT [128, 648]: 5 channel-pairs (108 cols) + 2 singles (54 cols)."""
    bands = _bands()
    t = np.zeros((128, TOEP_COLS), np.float32)
    for gi, (c0, nch, _src) in enumerate(GROUPS):
        for cl in range(nch):
            ch = c0 + cl
            for j in range(WJ):
                t[j:j + SIZE, GOFF[gi] + cl * WJ + j] = bands[ch]
    return t


def _chunks(total):
    out, off = [], 0
    while off < total:
        sz = min(CHUNK, total - off)
        out.append((off, sz))
        off += sz
    return out


# ---------------------------------------------------------------- device build
_CACHE = {}


def _apply_tile_patch():
    import concourse.tile as tile
    from concourse import mybir
    from concourse.vector_clock import ScopedClock

    def _patched(self, tick_clock, wait_clock):
        nc = self.nc
        drain_inst = nc.sync.drain()
        wait_clock.add_sem_waits(
            drain_inst.ins, ScopedClock({None: tick_clock.global_clock})
        )
        si = drain_inst.ins.sync_info
        if si is not None and si.on_wait and len(si.on_wait) > 1:
            extra = list(si.on_wait[1:])
            del si.on_wait[1:]
            for w in extra:
                d2 = nc.sync.drain()
                si2 = d2.ins.sync_info
                if si2 is None:
                    d2.ins.sync_info = mybir.SyncInfo(on_wait=[w], on_update=[])
                else:
                    si2.on_wait.append(w)
        nc.all_engine_barrier()
        popped = nc._tile_sem_poison_stack.pop()
        assert popped is self._sem_poison
        nc.clear_and_free_semaphores(list(self.sems.allocated().values()))
        nc.all_engine_barrier()

    tile.TileContext._drain_and_barrier = _patched


_WSPLIT_N = [0]


def _split_waits(nc):
    """This walrus build accepts only one sync-wait per instruction: hoist
    extra waits onto same-engine NoOps placed just before the instruction."""
    from concourse import mybir
    for f in nc.m.functions:
        for bb in f.blocks:
            new_list = []
            for ins in bb.instructions:
                si = getattr(ins, "sync_info", None)
                if si is not None and si.on_wait and len(si.on_wait) > 1:
                    extra = list(si.on_wait[:-1])
                    del si.on_wait[:-1]
                    for w in extra:
                        _WSPLIT_N[0] += 1
                        nop = mybir.InstDrain(
                            name=f"WSPLIT-{_WSPLIT_N[0]}",
                            engine=ins.engine,
                            sync_info=mybir.SyncInfo(on_wait=[w], on_update=[]),
                            bass_is_fusable=False,
                        )
                        new_list.append(nop)
                new_list.append(ins)
            bb.instructions[:] = new_list


def _build_main():
    import concourse.bass as bass
    import concourse.tile as tile
    from concourse import mybir
    from concourse.bass_types import AP

    f32 = mybir.dt.float32
    bf16 = mybir.dt.bfloat16
    Relu = mybir.ActivationFunctionType.Relu
    Ident = mybir.ActivationFunctionType.Identity
    Sqrt = mybir.ActivationFunctionType.Sqrt
    Square = mybir.ActivationFunctionType.Square
    Alu = mybir.AluOpType

    nc = bass.Bass("TRN2", num_devices=NCORES)
    # packed inputs: bf16 [fyp | fsp | toep(2 planes)] + small fp32 weights
    fin = nc.dram_tensor("fin", [2 * SW + 2, HPAD, W], bf16,
                         kind="ExternalInput")
    wtd = nc.dram_tensor("wtd", [WTD_SZ], f32, kind="ExternalInput")
    oo = nc.dram_tensor("oo", [SW, HI, W], f32, kind="ExternalOutput")
    fina = fin[:]
    wta = wtd[:]
    FYO = 0                      # fyp element offset (bf16 plane block)
    FSO = SW * HPAD * W          # fsp element offset
    O_TOEP = 2 * SW * HPAD * W   # toeplitz block offset (bf16)
    assert O_TOEP + 128 * TOEP_COLS <= (2 * SW + 2) * HPAD * W
    # fp32 weight sub-offsets inside wtd
    O_L1 = 0
    O_L2 = O_L1 + 9 * 12 * HID
    O_L3 = O_L2 + 9 * HID * HID
    O_B1 = O_L3 + 9 * HID
    O_B2 = O_B1 + 128
    O_B3 = O_B2 + 128
    assert O_B3 + NQ <= WTD_SZ

    with tile.TileContext(nc) as tc:
        with (
            tc.tile_pool(name="dram", bufs=1, space="DRAM") as dram,
            tc.tile_pool(name="singles", bufs=1) as singles,
            tc.tile_pool(name="stage", bufs=3) as stage,
            tc.tile_pool(name="stats", bufs=2) as statp,
            tc.tile_pool(name="io", bufs=2) as io,
            tc.tile_pool(name="acts", bufs=2) as acts,
            tc.tile_pool(name="psumA", bufs=2, space="PSUM") as psumA,
            tc.tile_pool(name="psumC", bufs=3, space="PSUM") as psumC,
            tc.tile_pool(name="psumS", bufs=1, space="PSUM") as psumS,
        ):
            cal = dram.tile([12, HREC, CALW], f32)
            ccin = dram.tile([1, 24], f32)
            ccout = dram.tile([1, 24], f32)

            # ---------------- load windows + weights
            fyw = singles.tile([128, SW, NWIN, W], bf16)
            fsw = singles.tile([128, SW, NWIN, W], bf16)
            for s in range(SW):
                for (dst, base) in ((fyw, FYO), (fsw, FSO)):
                    nc.sync.dma_start(
                        out=dst[:, s, :, :],
                        in_=AP(fina.tensor, base + s * HPAD * W,
                               [[W, 128], [WJ * W, NWIN], [1, W]]),
                    )
            toep_s = singles.tile([128, TOEP_COLS], bf16)
            nc.sync.dma_start(
                out=toep_s[:],
                in_=AP(fina.tensor, O_TOEP, [[TOEP_COLS, 128], [1, TOEP_COLS]]))

            w1s = singles.tile([48, 9, 128], f32)
            w2s = singles.tile([128, 9, 128], f32)
            w3s = singles.tile([128, 9, NQ], f32)
            nc.vector.memset(w1s[:], 0.0)
            nc.vector.memset(w2s[:], 0.0)
            nc.vector.memset(w3s[:], 0.0)
            for q in range(NQ):
                nc.sync.dma_start(
                    out=w1s[12 * q:12 * q + 12, :, 32 * q:32 * q + 32],
                    in_=AP(wta.tensor, O_L1,
                           [[HID, 12], [12 * HID, 9], [1, HID]]))
                nc.sync.dma_start(
                    out=w2s[32 * q:32 * q + 32, :, 32 * q:32 * q + 32],
                    in_=AP(wta.tensor, O_L2,
                           [[HID, HID], [HID * HID, 9], [1, HID]]))
                nc.sync.dma_start(
                    out=w3s[32 * q:32 * q + 32, :, q:q + 1],
                    in_=AP(wta.tensor, O_L3,
                           [[1, HID], [HID, 9], [1, 1]]))
            b1s = singles.tile([128, 1], f32)
            nc.sync.dma_start(out=b1s[:],
                              in_=AP(wta.tensor, O_B1, [[1, 128], [1, 1]]))
            b2s = singles.tile([128, 1], f32)
            nc.sync.dma_start(out=b2s[:],
                              in_=AP(wta.tensor, O_B2, [[1, 128], [1, 1]]))
            b3s = singles.tile([NQ, 1], f32)
            nc.sync.dma_start(out=b3s[:],
                              in_=AP(wta.tensor, O_B3, [[1, NQ], [1, 1]]))

            # ---------------- phase A: toeplitz matmuls -> cal DRAM
            for w in range(NWIN):
                for gi, (c0, nch, src) in enumerate(GROUPS):
                    st = fsw if src == 's' else fyw
                    ncols = nch * WJ
                    ps = psumA.tile([108, CALW], f32, tag="psA")
                    nc.tensor.matmul(
                        ps[:ncols, :],
                        lhsT=toep_s[:, GOFF[gi]:GOFF[gi] + ncols],
                        rhs=st[:, :, w, :], start=True, stop=True)
                    sg = stage.tile([108, CALW], f32, tag="stA")
                    nc.scalar.copy(sg[:ncols, :], ps[:ncols, :])
                    nc.sync.dma_start(
                        out=cal[c0:c0 + nch, WJ * w:WJ * w + WJ, :],
                        in_=sg[:ncols, :])

            # ---------------- BN stats: per-channel sum / sumsq
            sums2 = singles.tile([STP, 24], f32)
            nc.vector.memset(sums2[:], 0.0)
            scratch = singles.tile([STP, STF], f32)
            cala = cal[:]
            for ch in range(12):
                ct = statp.tile([STP, STF], f32, tag="ct")
                nc.sync.dma_start(
                    out=ct[:],
                    in_=AP(cala.tensor, cala.offset + ch * CALCH,
                           [[STF, STP], [1, STF]]))
                nc.vector.tensor_reduce(
                    out=sums2[:, ch:ch + 1], in_=ct[:],
                    axis=mybir.AxisListType.X, op=Alu.add)
                nc.scalar.activation(
                    out=scratch[:], in_=ct[:], func=Square,
                    accum_out=sums2[:, 12 + ch:12 + ch + 1])

            ones = singles.tile([STP, 1], f32)
            nc.vector.memset(ones[:], 1.0)
            pss = psumS.tile([1, 24], f32, tag="pstat")
            nc.tensor.matmul(pss[:], lhsT=ones[:], rhs=sums2[:],
                             start=True, stop=True)
            csb = singles.tile([1, 24], f32)
            nc.scalar.copy(csb[:], pss[:])
            nc.gpsimd.dma_start(out=ccin[:], in_=csb[:])
            if NOCC:
                nc.gpsimd.dma_start(out=ccout[:], in_=ccin[:])
            else:
                nc.gpsimd.collective_compute(
                    "AllReduce", Alu.add,
                    replica_groups=[list(range(NCORES))],
                    ins=[ccin[:].opt()], outs=[ccout[:].opt()])

            sum12 = singles.tile([12, 1], f32)
            sq12 = singles.tile([12, 1], f32)
            cca = ccout[:]
            nc.sync.dma_start(out=sum12[:],
                              in_=AP(cca.tensor, cca.offset, [[1, 12], [1, 1]]))
            nc.sync.dma_start(out=sq12[:],
                              in_=AP(cca.tensor, cca.offset + 12,
                                     [[1, 12], [1, 1]]))
            nglob = N_GLOB // NCORES if NOCC else N_GLOB
            mean12 = singles.tile([12, 1], f32)
            nc.vector.tensor_scalar_mul(mean12[:], sum12[:], 1.0 / nglob)
            ex2 = singles.tile([12, 1], f32)
            nc.vector.tensor_scalar_mul(ex2[:], sq12[:], 1.0 / nglob)
            m2 = singles.tile([12, 1], f32)
            nc.vector.tensor_tensor(out=m2[:], in0=mean12[:], in1=mean12[:],
                                    op=Alu.mult)
            var12 = singles.tile([12, 1], f32)
            nc.vector.tensor_tensor(out=var12[:], in0=ex2[:], in1=m2[:],
                                    op=Alu.subtract)
            nc.vector.tensor_scalar_add(var12[:], var12[:], BN_EPS)
            sd12 = singles.tile([12, 1], f32)
            nc.scalar.activation(out=sd12[:], in_=var12[:], func=Sqrt)
            rch12 = singles.tile([12, 1], f32)
            nc.vector.reciprocal(rch12[:], sd12[:])

            scale48 = singles.tile([48, 1], f32)
            mch48 = singles.tile([48, 1], f32)
            for q in range(NQ):
                nc.sync.dma_start(out=scale48[12 * q:12 * q + 12, :],
                                  in_=rch12[:])
                nc.sync.dma_start(out=mch48[12 * q:12 * q + 12, :],
                                  in_=mean12[:])

            # fold BN into conv1: w1f_scaled = w1s * rch ; b1e = b1 - sum(w1f_scaled * mch)
            w1sf = singles.tile([48, 9, 128], f32)
            nc.vector.tensor_scalar_mul(w1sf[:], w1s[:], scale48[:, 0:1])
            bc = singles.tile([1, 9 * 128], f32)
            for i in range(3):
                psb = psumS.tile([1, 384], f32, tag="pbc")
                nc.tensor.matmul(
                    psb[:],
                    lhsT=mch48[:],
                    rhs=w1sf[:].rearrange("p t o -> p (t o)")[:, 384 * i:384 * (i + 1)],
                    start=True, stop=True)
                nc.scalar.copy(bc[:, 384 * i:384 * (i + 1)], psb[:])
            bcr = singles.tile([1, 128], f32)
            bca = bc[:]
            nc.vector.tensor_reduce(
                out=bcr[:],
                in_=AP(bca.tensor, bca.offset, [bca.ap[0], [1, 128], [128, 9]]),
                axis=mybir.AxisListType.X, op=Alu.add)
            bct = singles.tile([128, 1], f32)
            nc.sync.dma_start(out=bct[:], in_=bcr[:])
            b1e = singles.tile([128, 1], f32)
            nc.vector.tensor_tensor(out=b1e[:], in0=b1s[:], in1=bct[:],
                                    op=Alu.subtract)

            # ---------------- conv stations
            for st_i in range(NST):
                sw, t_i = st_i // NT, st_i % NT
                calt = io.tile([48, CAL_SZ], f32, tag="cal")
                nc.vector.memset(calt[:], 0.0)
                nc.vector.tensor_scalar_add(calt[:], calt[:], mch48[:, 0:1])
                calr = calt[:, 1:1 + CAL_F].rearrange(
                    "p (r x) -> p r x", x=W2)
                for q in range(NQ):
                    r0 = QROWS * q + R * t_i - 3
                    lo, hi = max(r0, 0), min(r0 + CAL_ROWS, HI)
                    nc.sync.dma_start(
                        out=calr[12 * q:12 * q + 12, lo - r0:hi - r0, 1:53],
                        in_=AP(cala.tensor, cala.offset + lo * CALW + sw * W,
                               [[CALCH, 12], [CALW, hi - lo], [1, W]]))

                h1 = acts.tile([128, H1_SZ], f32, tag="h1")
                h2 = acts.tile([128, H2_SZ], f32, tag="h2")
                ot = io.tile([NQ, O_F], f32, tag="ot")
                nc.vector.memset(h1[:], 0.0)

                # ---- conv1 (BN folded): cal[48] -> h1[128], ReLU(. + b1e)
                for off, sz in _chunks(H1_F):
                    ps = psumC.tile([128, CHUNK], f32, tag="ps")
                    for t9 in range(9):
                        dy, dx = t9 // 3 - 1, t9 % 3 - 1
                        base = off + W2 * (1 + dy) + dx + 1
                        nc.tensor.matmul(
                            ps[:, :sz], lhsT=w1sf[:, t9, :],
                            rhs=calt[:, base:base + sz],
                            start=(t9 == 0), stop=(t9 == 8))
                    nc.scalar.activation(
                        out=h1[:, 1 + off:1 + off + sz], in_=ps[:, :sz],
                        func=Relu, bias=b1e[:, 0:1], scale=1.0)
                h1v = h1[:, 1:1 + H1_F].rearrange("p (r c) -> p r c", c=W2)
                nc.vector.memset(h1v[:, :, 0:1], 0.0)
                nc.vector.memset(h1v[:, :, W2 - 1:W2], 0.0)
                if t_i == 0:
                    nc.vector.memset(h1[0:32, 1:1 + 2 * W2], 0.0)
                if t_i == NT - 1:
                    nc.vector.memset(
                        h1[96:128, 1 + (H1_ROWS - 2) * W2:1 + H1_F], 0.0)

                # ---- conv2: h1[128] -> h2[128], ReLU(. + b2)
                nc.vector.memset(h2[:], 0.0)
                for off, sz in _chunks(H2_F):
                    ps = psumC.tile([128, CHUNK], f32, tag="ps")
                    for t9 in range(9):
                        dy, dx = t9 // 3 - 1, t9 % 3 - 1
                        base = off + W2 * (1 + dy) + dx + 1
                        nc.tensor.matmul(
                            ps[:, :sz], lhsT=w2s[:, t9, :],
                            rhs=h1[:, base:base + sz],
                            start=(t9 == 0), stop=(t9 == 8))
                    nc.scalar.activation(
                        out=h2[:, 1 + off:1 + off + sz], in_=ps[:, :sz],
                        func=Relu, bias=b2s[:, 0:1], scale=1.0)
                h2v = h2[:, 1:1 + H2_F].rearrange("p (r c) -> p r c", c=W2)
                nc.vector.memset(h2v[:, :, 0:1], 0.0)
                nc.vector.memset(h2v[:, :, W2 - 1:W2], 0.0)
                if t_i == 0:
                    nc.vector.memset(h2[0:32, 1:1 + W2], 0.0)
                if t_i == NT - 1:
                    nc.vector.memset(
                        h2[96:128, 1 + (H2_ROWS - 1) * W2:1 + H2_F], 0.0)

                # ---- conv3: h2[128] -> o[4], Identity(. + b3 + c)
                for off, sz in _chunks(O_F):
                    ps = psumS.tile([NQ, CHUNK], f32, tag="ps3")
                    for t9 in range(9):
                        dy, dx = t9 // 3 - 1, t9 % 3 - 1
                        base = off + W2 * (1 + dy) + dx + 1
                        nc.tensor.matmul(
                            ps[:, :sz], lhsT=w3s[:, t9, :],
                            rhs=h2[:, base:base + sz],
                            start=(t9 == 0), stop=(t9 == 8))
                    nc.scalar.activation(
                        out=ot[:, off:off + sz], in_=ps[:, :sz],
                        func=Ident, bias=b3s[:, 0:1], scale=1.0)

                # ---- + fs_sel, DMA out
                fst = io.tile([NQ, R * W], bf16, tag="fst")
                nc.sync.dma_start(
                    out=fst[:],
                    in_=AP(fina.tensor,
                           FSO + sw * HPAD * W + (HALF + R * t_i) * W,
                           [[QROWS * W, NQ], [W, R], [1, W]]))
                otr = ot[:].rearrange("p (r x) -> p r x", x=W2)
                fstr = fst[:].rearrange("p (r x) -> p r x", x=W)
                nc.vector.tensor_tensor(out=otr[:, :, 1:53],
                                        in0=otr[:, :, 1:53],
                                        in1=fstr[:], op=Alu.add)
                ooa = oo[:]
                nc.sync.dma_start(
                    out=AP(ooa.tensor, sw * HI * W + R * t_i * W,
                           [[QROWS * W, NQ], [W, R], [1, W]]),
                    in_=otr[:, :, 1:53])
    if not SIM:
        _split_waits(nc)
    return nc


# ---------------------------------------------------------------- run
def _get_main():
    if "nc" not in _CACHE:
        _apply_tile_patch()
        _CACHE["nc"] = _build_main()
    return _CACHE["nc"]


def _make_fast(nc):
    """Cached jit of the same program run_bass_via_pjrt traces per call:
    saves the per-call retrace, and makes the donated zero output buffer
    on-device instead of shipping 5.5 MB of host zeros every launch."""
    import jax
    import jax.numpy as jnp
    from jax.sharding import Mesh, PartitionSpec, NamedSharding
    from jax.experimental.shard_map import shard_map
    from concourse import mybir
    from concourse.bass2jax import (
        _bass_exec_p, partition_id_tensor, install_neuronx_cc_hook)

    install_neuronx_cc_hook()
    pname = nc.partition_id_tensor.name if nc.partition_id_tensor else None
    in_names, out_names, out_avals = [], [], []
    for alloc in nc.m.functions[0].allocations:
        if not isinstance(alloc, mybir.MemoryLocationSet):
            continue
        name = alloc.memorylocations[0].name
        if alloc.kind == "ExternalInput":
            if name != pname:
                in_names.append(name)
        elif alloc.kind == "ExternalOutput":
            out_avals.append(jax.core.ShapedArray(
                tuple(alloc.tensor_shape), mybir.dt.np(alloc.dtype)))
            out_names.append(name)
    assert in_names == ["fin", "wtd"] and out_names == ["oo"]
    n_params, n_outs = 2, 1
    all_names = in_names + out_names + ([pname] if pname else [])
    donate = tuple(range(n_params, n_params + n_outs))

    def _body(*args):
        operands = list(args)
        if pname is not None:
            operands.append(partition_id_tensor())
        outs = _bass_exec_p.bind(
            *operands,
            out_avals=tuple(out_avals),
            in_names=tuple(all_names),
            out_names=tuple(out_names),
            lowering_input_output_aliases=(),
            sim_require_finite=True,
            sim_require_nnan=True,
            nc=nc,
        )
        return tuple(outs)

    devices = jax.devices()[:NCORES]
    mesh = Mesh(np.asarray(devices), ("core",))
    sharded = jax.jit(
        shard_map(_body, mesh=mesh,
                  in_specs=(PartitionSpec("core"),) * 3,
                  out_specs=(PartitionSpec("core"),),
                  check_rep=False),
        donate_argnums=donate, keep_unused=True)
    zsh = NamedSharding(mesh, PartitionSpec("core"))
    zeros_fn = jax.jit(
        lambda: jnp.zeros((NCORES * SW, HI, W), jnp.float32),
        out_shardings=zsh)
    return sharded, zeros_fn


def _run_fast(global_fin, global_wtd):
    """[NCORES*(2*SW+2), HPAD, W] bf16 + [NCORES*WTD_SZ] f32 -> out."""
    sharded, zeros_fn = _CACHE["fast"]
    out, = sharded(global_fin, global_wtd, zeros_fn())
    return np.asarray(out)


def _warmup():
    """Build + compile + one dummy execution at import: warms the walrus/jax
    compile caches, the PJRT client, the device programs and the collective
    comm so the first real kernel() call runs at steady-state speed."""
    if _CACHE.get("warm"):
        return
    try:
        import jax
        try:
            jax.config.update("jax_compilation_cache_dir",
                              "/root/.jax_bass_cache")
            jax.config.update("jax_persistent_cache_min_entry_size_bytes", -1)
            jax.config.update("jax_persistent_cache_min_compile_time_secs", 0)
        except Exception:
            pass
        nc = _get_main()
        from concourse.bass_utils import run_bass_kernel_spmd
        import ml_dtypes
        bf = ml_dtypes.bfloat16
        rng = np.random.default_rng(7)
        dums = [rng.standard_normal((2 * SW + 2, HPAD, W)).astype(bf)
                for _ in range(NCORES)]
        wds = [rng.standard_normal((WTD_SZ,)).astype(np.float32)
               for _ in range(NCORES)]
        ref = run_bass_kernel_spmd(
            nc, [dict(fin=d, wtd=w) for d, w in zip(dums, wds)],
            core_ids=list(range(NCORES)))
        ref_out = np.concatenate([r["oo"] for r in ref.results], axis=0)
        try:
            _CACHE["fast"] = _make_fast(nc)
            fast_out = _run_fast(np.concatenate(dums, axis=0),
                                 np.concatenate(wds, axis=0))
            if not np.array_equal(fast_out, ref_out):
                del _CACHE["fast"]
        except Exception:
            _CACHE.pop("fast", None)
        _CACHE["warm"] = True
    except Exception as e:  # warmup is best-effort only
        import logging
        logging.getLogger(__name__).warning(f"kernel warmup skipped: {e}")


def _run(in_maps):
    nc = _get_main()
    import time as _time
    t0 = _time.time()
    if SIM:
        from concourse.bass_interp import MultiCoreSim
        sim = MultiCoreSim(nc, num_cores=NCORES)
        for c, cs in sim.cores.items():
            for k, v in in_maps[c].items():
                cs.tensor(k)[:] = v
        sim.simulate(check_with_hw=False)
        res = [{"oo": np.array(sim.cores[c].tensor("oo"))}
               for c in range(NCORES)]
    else:
        from concourse.bass_utils import run_bass_kernel_spmd
        r = run_bass_kernel_spmd(nc, in_maps, core_ids=list(range(NCORES)))
        res = r.results
        if r.exec_time_ns is not None:
            _CACHE.setdefault("exec_ns", {})["m"] = r.exec_time_ns
    _CACHE.setdefault("wall_ns", {})["m"] = int((_time.time() - t0) * 1e9)
    return res


# ---------------------------------------------------------------- main entry
def kernel(sv_uncal, sv_bg, kernel, w1, b1, w2, b2, w3, b3, msk_idx, row_idx):
    sv_uncal = np.asarray(sv_uncal, np.float32)
    sv_bg = np.asarray(sv_bg, np.float32)
    w1 = np.asarray(w1, np.float32)
    b1 = np.asarray(b1, np.float32)
    w2 = np.asarray(w2, np.float32)
    b2 = np.asarray(b2, np.float32)
    w3 = np.asarray(w3, np.float32)
    b3 = np.asarray(b3, np.float32)
    msk_idx = np.asarray(msk_idx)
    row_idx = np.asarray(row_idx)

    # ---- host gather + replicate/zero pad
    fy = sv_uncal.reshape(B * P, H, W)[msk_idx][:, row_idx]   # [24, 1100, 52]
    fs = sv_bg.reshape(B * P, H, W)[msk_idx][:, row_idx]
    fyp = np.zeros((M_SEL, HPAD, W), np.float32)
    fsp = np.zeros((M_SEL, HPAD, W), np.float32)
    fyp[:, :HALF + HI + HALF] = np.pad(
        fy, ((0, 0), (HALF, HALF), (0, 0)), mode="edge")
    fsp[:, :HALF + HI + HALF] = np.pad(
        fs, ((0, 0), (HALF, HALF), (0, 0)), mode="edge")

    # ---- constant device weights, packed into 2 trailing fin planes
    w1f = np.concatenate(
        [w1[:, 0:10] + w1[:, 11:21], w1[:, 10:11], w1[:, 21:22]], axis=1)
    l1c = w1f.transpose(2, 3, 1, 0).reshape(9, 12, HID)     # [t9, ch, o]
    l2c = w2.transpose(2, 3, 1, 0).reshape(9, HID, HID)     # [t9, i, o]
    l3c = w3[0].transpose(1, 2, 0).reshape(9, HID, 1)       # [t9, i, 1]
    b1t = np.tile(b1, NQ).astype(np.float32)
    b2t = np.tile(b2, NQ).astype(np.float32)
    b3t = np.full((NQ,), b3[0] + np.float32(NS[0] / NS[1]), np.float32)
    import ml_dtypes
    bf = ml_dtypes.bfloat16
    wts = np.zeros((WTD_SZ,), np.float32)
    flat = np.concatenate([
        l1c.ravel(), l2c.ravel(), l3c.ravel(), b1t, b2t, b3t])
    wts[:flat.size] = flat
    tplanes = np.zeros((2 * HPAD * W,), bf)
    tplanes[:128 * TOEP_COLS] = _toep_packed().ravel().astype(bf)
    tplanes = tplanes.reshape(2, HPAD, W)

    nplanes = 2 * SW + 2
    gfin = np.empty((NCORES * nplanes, HPAD, W), bf)
    for c in range(NCORES):
        gfin[c * nplanes:c * nplanes + SW] = fyp[SW * c:SW * c + SW]
        gfin[c * nplanes + SW:c * nplanes + 2 * SW] = fsp[SW * c:SW * c + SW]
        gfin[c * nplanes + 2 * SW:(c + 1) * nplanes] = tplanes
    gwtd = np.tile(wts, NCORES)

    # Device outputs are deterministic; rare transient corruption (dropped
    # DMA -> zero blocks, NaNs) is detected cheaply and the launch retried.
    import time as _time
    for _attempt in range(3):
        if "fast" in _CACHE:
            t0 = _time.time()
            out = _run_fast(gfin, gwtd)                       # [24, 1100, 52]
            w = _CACHE.setdefault("wall_ns", {})
            w["m"] = w.get("m", 0) + int((_time.time() - t0) * 1e9)
        else:
            in_maps = [dict(fin=gfin[c * nplanes:(c + 1) * nplanes],
                            wtd=wts)
                       for c in range(NCORES)]
            res = _run(in_maps)
            out = np.concatenate([r["oo"] for r in res], axis=0)
        if (np.isfinite(out).all()
                and np.count_nonzero(out == 0.0) <= 64
                and np.abs(out).max() < 1e3):
            break
    out_cal = np.zeros((B * P, HI, W), np.float32)
    np.add.at(out_cal, msk_idx, out)
    cnt = np.zeros((B * P,), np.float32)
    np.add.at(cnt, msk_idx, 1.0)
    out_msk = np.broadcast_to(
        (cnt > 0)[:, None, None], (B * P, HI, W)).copy()
    return (out_cal.reshape(B, P, HI, W),
            out_msk.reshape(B, P, HI, W))


import os as _os
if not _os.environ.get("SIM") and not _os.environ.get("NO_WARMUP"):
    _warmup()
